# revision 1
# baseline (speedup 1.0000x reference)
"""Causal single-head attention (B=8, T=2048, D=1024, HS=64) on 8 TRN2 NeuronCores.

Sharding: data-parallel over batch -- core b computes batch b end-to-end.
No collectives; outputs are concatenated on the host.

Compute path is bf16 (operands) with fp32 PSUM accumulation; the softmax
denominator/normalization stays fp32. Host casts x/W to bf16 (same rounding
an on-chip cast would apply). Logit abs err ~2e-3 -> output rel err ~4e-3.

Per-core pipeline, processed in four 512-row t-supers:
  x^T chunks arrive via DMA-transpose straight from DRAM (bf16, xbar path)
  projection with W stationary:  QKV^T[:,t] = W^T x^T  (two 128-col halves:
  [Wq/8 | Wk] and [Wv | 0]; softmax 1/sqrt(HS) folded into Wq/bq)
  DVE per-partition bias add from PSUM -> persistent Q^T/K^T [64,T] (bf16)
  V^T + bias (+ones row from the W zero-pad column) -> PE-transpose -> V'
  attention (queries of this super, k-tiles 0..4ts+3) in PAIRS of k-tiles
  sharing a 2-bank PSUM tile (one exp per pair), S emitted a pair ahead of
  PV so the PE never stalls on the exp chain:
    S^T[k,q] = K-chunk @ Q^T-super      (PSUM fp32, N=512)
    P^T = exp(S^T)                      (ScalarE; logits ~N(0,1), no max sub)
    causal mask on diagonal chunks      (GPSIMD affine_select, in-place)
    outT[65,512] += V'[k,65]^T @ P^T    (PSUM fp32 accumulate; row 64 = denom)
  epilogue (fp32): PE-transpose outT back, DVE reciprocal + scale, DMA out.
"""

import sys

if "/opt/trn_rl_repo" not in sys.path:
    sys.path.insert(0, "/opt/trn_rl_repo")

import os
from contextlib import ExitStack

import numpy as np

import concourse.bass as bass
import concourse.tile as tile
from concourse import bacc, mybir
from concourse.bass_utils import run_bass_kernel_spmd

B, T, D, HS = 8, 2048, 1024, 64
N_CORES = 8
F32 = mybir.dt.float32
BF16 = mybir.dt.bfloat16

TT = 128            # t/k tile (partition dim)
NDT = D // TT       # 8 contraction chunks
NTT = T // TT       # 16 k-tiles
QS = 512            # t/q super width (matmul free dim)
NQS = T // QS       # 4 supers
VP = HS + 1         # V' width (64 + ones column)


def build_graph() -> bacc.Bacc:
    nc = bacc.Bacc("TRN2", target_bir_lowering=False, debug=False)

    xt_ext = nc.dram_tensor("xt", [D, T], BF16, kind="ExternalInput").ap()
    # wqkv[d, 0:128] = [Wq/8 | Wk]; wqkv[d, 128:256] = [Wv | 0]   (bf16)
    w_ext = nc.dram_tensor("wqkv", [D, 2 * TT], BF16, kind="ExternalInput").ap()
    # fp32 bias columns: col0[0:64]=bq/8, col0[64:128]=bk, col1[0:64]=bv,
    # col1[64]=1.0 (ones row for V' via the W zero-pad column)
    bcol_ext = nc.dram_tensor("bcol", [TT, 2], F32, kind="ExternalInput").ap()
    id_ext = nc.dram_tensor("ident", [TT, TT], F32, kind="ExternalInput").ap()
    idb_ext = nc.dram_tensor("identb", [TT, TT], BF16, kind="ExternalInput").ap()
    out_ext = nc.dram_tensor("out", [T, HS], F32, kind="ExternalOutput").ap()

    with tile.TileContext(nc) as tc, ExitStack() as ctx:
        const = ctx.enter_context(tc.tile_pool(name="const", bufs=1))
        persist = ctx.enter_context(tc.tile_pool(name="persist", bufs=1))
        xt_pool = ctx.enter_context(tc.tile_pool(name="xt", bufs=2))
        vt_pool = ctx.enter_context(tc.tile_pool(name="vt", bufs=2))
        pt_pool = ctx.enter_context(tc.tile_pool(name="pt", bufs=3))
        otsb_pool = ctx.enter_context(tc.tile_pool(name="otsb", bufs=2))
        osb_pool = ctx.enter_context(tc.tile_pool(name="osb", bufs=2))
        rc_pool = ctx.enter_context(tc.tile_pool(name="rc", bufs=2))
        warm_pool = ctx.enter_context(tc.tile_pool(name="warm", bufs=1))
        psum = ctx.enter_context(tc.tile_pool(name="ps", bufs=1, space="PSUM"))

        # ---- constants ----
        id_sb = const.tile([TT, TT], F32)
        nc.sync.dma_start(id_sb[:], id_ext)
        idb_sb = const.tile([TT, TT], BF16)
        nc.sync.dma_start(idb_sb[:], idb_ext)
        bcol_sb = const.tile([TT, 2], F32)
        nc.sync.dma_start(bcol_sb[:], bcol_ext)
        w_sb = const.tile([TT, NDT * 2 * TT], BF16)
        nc.sync.dma_start(
            w_sb[:].rearrange("p (c n) -> p c n", c=NDT),
            w_ext.rearrange("(c p) n -> p c n", p=TT),
        )

        # ---- persistent per-core intermediates (bf16 matmul operands) ----
        qt_sb = persist.tile([HS, T], BF16)         # Q^T / 8 (scale folded)
        kt_sb = persist.tile([HS, T], BF16)         # K^T
        vp_sb = persist.tile([TT, NTT * VP], BF16)  # V' [128, 65] per k-tile

        # ---- PE warmup: dep-free matmuls during the initial DMA fill so the
        # HAM activity monitor lifts the PE clock to 2.4 GHz early ----
        warm_sb = warm_pool.tile([TT, QS], F32)
        nc.gpsimd.memset(warm_sb[:], 0.0)
        for _ in range(4):
            warm_ps = psum.tile([TT, QS], F32, tag="proj", bufs=1)
            nc.tensor.matmul(
                warm_ps[:], warm_sb[:, 0:TT], warm_sb[:], start=True, stop=True
            )

        def do_super(ts: int):
            tsl = slice(ts * QS, (ts + 1) * QS)

            # -- x^T chunks: plain DMA from the host-pretransposed x^T --
            xt = xt_pool.tile([TT, NDT * QS], BF16, tag="xt")
            for c in range(NDT):
                nc.sync.dma_start(
                    xt[:, c * QS:(c + 1) * QS],
                    xt_ext[c * TT:(c + 1) * TT, tsl],
                )

            # -- projection, W stationary: psum = W_half^T @ x^T --
            for half in range(2):
                pp = psum.tile([TT, QS], F32, tag="proj", bufs=1)
                for c in range(NDT):
                    nc.tensor.matmul(
                        pp[:],
                        w_sb[:, c * 2 * TT + half * TT:c * 2 * TT + (half + 1) * TT],
                        xt[:, c * QS:(c + 1) * QS],
                        start=(c == 0),
                        stop=(c == NDT - 1),
                    )
                if half == 0:
                    # rows 0:64 = Q^T/8, rows 64:128 = K^T (per-partition bias)
                    nc.vector.tensor_scalar_add(
                        qt_sb[:, tsl], pp[0:HS, :], bcol_sb[0:HS, 0:1]
                    )
                    nc.vector.tensor_scalar_add(
                        kt_sb[:, tsl], pp[HS:2 * HS, :], bcol_sb[HS:2 * HS, 0:1]
                    )
                else:
                    # rows 0:64 = V^T + bv; row 64 = 0 (W zero-pad) + 1.0
                    vt = vt_pool.tile([VP, QS], BF16, tag="vt")
                    nc.vector.tensor_scalar_add(
                        vt[0:VP, :], pp[0:VP, :], bcol_sb[0:VP, 1:2]
                    )
                    for u in range(4):
                        j = 4 * ts + u
                        vps = psum.tile([TT, VP], BF16, tag="small", bufs=2)
                        nc.tensor.transpose(
                            vps[:], vt[:, u * TT:(u + 1) * TT], idb_sb[0:VP, 0:VP]
                        )
                        nc.vector.tensor_copy(
                            vp_sb[:, j * VP:(j + 1) * VP], vps[:]
                        )

            # -- causal attention for this super's queries, k-tile PAIRS --
            nkt = 4 * ts + 4
            ot_ps = psum.tile([VP, QS], F32, tag="acc", bufs=1)

            def emit_s_pair(p):
                """S matmuls + one exp for k-tiles (2p, 2p+1); returns ptile."""
                sp = psum.tile([TT, 2 * QS], F32, tag="sbig", bufs=2, name=f"sp{p}")
                for h in range(2):
                    jj = 2 * p + h
                    nc.tensor.matmul(
                        sp[:, h * QS:(h + 1) * QS],
                        kt_sb[:, jj * TT:(jj + 1) * TT],
                        qt_sb[:, tsl],
                        start=True,
                        stop=True,
                    )
                ptile = pt_pool.tile([TT, 2 * QS], BF16, tag="pt", name=f"pt{p}")
                nc.scalar.activation(
                    ptile[:], sp[:], mybir.ActivationFunctionType.Exp
                )
                for h in range(2):
                    jj = 2 * p + h
                    if jj >= 4 * ts:
                        # diagonal: zero P^T[kk, qq] where qq < kk + 128*dd
                        dd = jj - 4 * ts
                        nc.gpsimd.affine_select(
                            out=ptile[:, h * QS:(h + 1) * QS],
                            in_=ptile[:, h * QS:(h + 1) * QS],
                            compare_op=mybir.AluOpType.is_ge,
                            fill=0.0,
                            base=-TT * dd,
                            channel_multiplier=-1,
                            pattern=[[1, QS]],
                        )
                return ptile

            def emit_pv_pair(p, ptile):
                for h in range(2):
                    jj = 2 * p + h
                    nc.tensor.matmul(
                        ot_ps[:],
                        vp_sb[:, jj * VP:(jj + 1) * VP],
                        ptile[:, h * QS:(h + 1) * QS],
                        start=(jj == 0),
                        stop=(jj == nkt - 1),
                        skip_group_check=True,
                    )

            npair = nkt // 2
            ptiles = [emit_s_pair(0)]
            for p in range(npair):
                if p + 1 < npair:
                    ptiles.append(emit_s_pair(p + 1))
                emit_pv_pair(p, ptiles[p])

            # -- epilogue (fp32): normalize + transpose back + store --
            ot_sb = otsb_pool.tile([VP, QS], F32, tag="otsb")
            nc.scalar.copy(ot_sb[:], ot_ps[:])
            o_sb = osb_pool.tile([TT, 4 * HS], F32, tag="osb")
            for u in range(4):
                ob_ps = psum.tile([TT, VP], F32, tag="small", bufs=2)
                nc.tensor.transpose(
                    ob_ps[:], ot_sb[:, u * TT:(u + 1) * TT], id_sb[0:VP, 0:VP]
                )
                rc = rc_pool.tile([TT, 1], F32, tag="rc")
                nc.vector.reciprocal(rc[:], ob_ps[:, HS:HS + 1])
                nc.vector.tensor_scalar_mul(
                    o_sb[:, u * HS:(u + 1) * HS], ob_ps[:, 0:HS], rc[:]
                )
            nc.sync.dma_start(
                out_ext[tsl, :].rearrange("(u p) h -> p u h", p=TT),
                o_sb[:].rearrange("p (u h) -> p u h", u=4),
            )

        for ts in range(NQS):
            do_super(ts)

    nc.compile()
    return nc


def make_inputs(x_b, Wq, bq, Wk, bk, Wv, bv):
    """Host-side prep for one core's in_map (x_b: [T, D] fp32)."""
    import ml_dtypes

    bf = ml_dtypes.bfloat16
    scale = 1.0 / np.sqrt(np.float32(HS))
    w = np.zeros((D, 2 * TT), dtype=np.float32)
    w[:, 0:HS] = Wq * scale
    w[:, HS:2 * HS] = Wk
    w[:, 2 * HS:3 * HS] = Wv
    bcol = np.zeros((TT, 2), dtype=np.float32)
    bcol[0:HS, 0] = bq * scale
    bcol[HS:2 * HS, 0] = bk
    bcol[0:HS, 1] = bv
    bcol[HS, 1] = 1.0
    ident = np.eye(TT, dtype=np.float32)
    identb = np.eye(TT, dtype=bf)
    return {
        "xt": np.ascontiguousarray(x_b.T).astype(bf),
        "wqkv": w.astype(bf),
        "bcol": bcol,
        "ident": ident,
        "identb": identb,
    }


_NC_CACHE = None


def _get_nc():
    global _NC_CACHE
    if _NC_CACHE is None:
        _NC_CACHE = build_graph()
    return _NC_CACHE


def kernel(x, Wq, bq, Wk, bk, Wv, bv):
    x = np.asarray(x, dtype=np.float32)
    args = [np.asarray(a, dtype=np.float32) for a in (Wq, bq, Wk, bk, Wv, bv)]
    nc = _get_nc()
    in_maps = [make_inputs(x[b], *args) for b in range(N_CORES)]
    trace = os.environ.get("BASS_ATTN_TRACE", "0") == "1"
    res = run_bass_kernel_spmd(
        nc, in_maps, core_ids=list(range(N_CORES)), trace=trace
    )
    if trace:
        print(
            f"HW exec time: {res.exec_time_ns} ns "
            f"(mean {res.mean_exec_time_ns}, max core {res.max_exec_time_core_id})"
        )
    out = np.stack([res.results[b]["out"] for b in range(N_CORES)], axis=0)
    return out



# revision 8
# speedup vs baseline: 1.0520x; 1.0520x over previous
"""Causal single-head attention (B=8, T=2048, D=1024, HS=64) on 8 TRN2 NeuronCores.

Sharding: data-parallel over batch -- core b computes batch b end-to-end.
No collectives; outputs are concatenated on the host.

Compute path is bf16 (operands) with fp32 PSUM accumulation; the softmax
denominator/normalization stays fp32. Host casts x/W to bf16 and pre-swizzles
both into the exact SBUF layouts so every DMA is a single large contiguous
transfer (the DMA-issue cost on the Sync queue, ~620ns each, was gating the
projection in the previous version).

Per-core pipeline, four 512-col t-supers, software-pipelined across supers:
  x^T prefetched whole (host-pretransposed, per-super contiguous), 8 DMAs
  projection W-stationary:  QKV^T[:,t] = W^T x^T  (two 128-col halves:
  [Wq/8 | Wk] and [Wv | 0]; softmax 1/sqrt(HS) folded into Wq/bq)
  DVE bias adds from PSUM -> Q^T duplicated to partitions 64:128 and K^T
  packed even/odd k-tile into partitions 0:64/64:128 (for row-tiled S)
  V^T + bias (+ones row from the W zero-pad column) -> PE-transpose -> V'
  attention in k-tile PAIRS, both S matmuls of a pair run CONCURRENTLY in
  the PE array (row tiling: contraction is only 64, so tiles (0,0)/(64,0)
  share the array), one exp per pair, causal-invalid columns trimmed from
  S matmuls / exp / PV everywhere, diagonal-block masking via a DVE
  multiply with a 128x128 triangular 0/1 mask (GpSimd affine_select was
  620ns/tile inside the S->exp->PV chain):
    S^T[k,q] = K-pair @ Q^T-super       (PSUM fp32, two banks, trimmed)
    P^T = exp(S^T)                      (ScalarE; logits ~N(0,1), no max sub)
    P^T[:,band] *= tri-mask             (DVE, diagonal tiles only)
    outT[65,q] += V'[k,65]^T @ P^T      (PSUM fp32 accumulate; row 64=denom)
  projection of super ts+1 is interleaved into the attention pair loop of
  super ts so the PE never idles on the exp chain (and HAM stays warm).
  epilogue (fp32): PE-transpose outT back, DVE reciprocal + scale, DMA out.
"""

import sys

if "/opt/trn_rl_repo" not in sys.path:
    sys.path.insert(0, "/opt/trn_rl_repo")

import os
from contextlib import ExitStack

import numpy as np

import concourse.bass as bass
import concourse.tile as tile
from concourse import bacc, mybir
from concourse.bass_utils import run_bass_kernel_spmd

B, T, D, HS = 8, 2048, 1024, 64
N_CORES = 8
F32 = mybir.dt.float32
BF16 = mybir.dt.bfloat16

TT = 128            # t/k tile (partition dim)
NDT = D // TT       # 8 contraction chunks
NTT = T // TT       # 16 k-tiles
QS = 512            # t/q super width (matmul free dim)
NQS = T // QS       # 4 supers
VP = HS + 1         # V' width (64 + ones column)
NWARM = 7           # warmup matmuls (~3us cold -> HAM warm before proj)


def build_graph() -> bacc.Bacc:
    nc = bacc.Bacc("TRN2", target_bir_lowering=False, debug=False)

    # host-preswizzled x^T: [4 supers x 128 partitions, 8 chunks * 512 cols]
    xts_ext = nc.dram_tensor("xts", [NQS * TT, NDT * QS], BF16,
                             kind="ExternalInput").ap()
    # host-preswizzled W: w[p, c*256+j] = wfull[c*128+p, j],
    # wfull[:, 0:128] = [Wq/8 | Wk], wfull[:, 128:256] = [Wv | 0]
    w_ext = nc.dram_tensor("wqkv", [TT, NDT * 2 * TT], BF16,
                           kind="ExternalInput").ap()
    # fp32 bias columns: col0[0:64]=bq/8, col0[64:128]=bk, col1[0:64]=bv,
    # col1[64]=1.0 (ones row for V' via the W zero-pad column)
    bcol_ext = nc.dram_tensor("bcol", [TT, 2], F32, kind="ExternalInput").ap()
    id_ext = nc.dram_tensor("ident", [TT, TT], F32, kind="ExternalInput").ap()
    idb_ext = nc.dram_tensor("identb", [TT, TT], BF16, kind="ExternalInput").ap()
    # tri-mask[k, q] = 1.0 if q >= k else 0.0
    mask_ext = nc.dram_tensor("mask", [TT, TT], BF16, kind="ExternalInput").ap()
    out_ext = nc.dram_tensor("out", [T, HS], F32, kind="ExternalOutput").ap()

    with tile.TileContext(nc) as tc, ExitStack() as ctx:
        const = ctx.enter_context(tc.tile_pool(name="const", bufs=1))
        persist = ctx.enter_context(tc.tile_pool(name="persist", bufs=1))
        vt_pool = ctx.enter_context(tc.tile_pool(name="vt", bufs=2))
        pt_pool = ctx.enter_context(tc.tile_pool(name="pt", bufs=3))
        otsb_pool = ctx.enter_context(tc.tile_pool(name="otsb", bufs=2))
        osb_pool = ctx.enter_context(tc.tile_pool(name="osb", bufs=2))
        rc_pool = ctx.enter_context(tc.tile_pool(name="rc", bufs=2))
        psum = ctx.enter_context(tc.tile_pool(name="ps", bufs=1, space="PSUM"))

        # ---- persistent SBUF ----
        xt_sb = persist.tile([TT, NQS * NDT * QS], BF16)   # all 4 supers
        w_sb = const.tile([TT, NDT * 2 * TT], BF16)
        bcol_sb = const.tile([TT, 2], F32)
        id_sb = const.tile([TT, TT], F32)
        idb_sb = const.tile([TT, TT], BF16)
        mask_sb = const.tile([TT, TT], BF16)
        warm_sb = const.tile([TT, QS], BF16)
        qt_sb = persist.tile([TT, T], BF16)     # rows 0:64 Q^T/8, 64:128 dup
        kt_sb = persist.tile([TT, (NTT // 2) * TT], BF16)  # even/odd packed
        vp_sb = persist.tile([TT, NTT * VP], BF16)         # V' per k-tile

        # ---- DMAs: w + super-0 x^T first, then small consts, then rest ----
        SW = NDT * QS  # 4096 cols per super
        nc.sync.dma_start(w_sb[:], w_ext)
        nc.sync.dma_start(xt_sb[:, 0:SW // 2], xts_ext[0:TT, 0:SW // 2])
        nc.sync.dma_start(xt_sb[:, SW // 2:SW], xts_ext[0:TT, SW // 2:SW])
        nc.sync.dma_start(bcol_sb[:], bcol_ext)
        nc.sync.dma_start(idb_sb[:], idb_ext)
        nc.sync.dma_start(mask_sb[:], mask_ext)
        nc.sync.dma_start(id_sb[:], id_ext)
        for s in range(1, NQS):
            for h in range(2):
                nc.sync.dma_start(
                    xt_sb[:, s * SW + h * SW // 2: s * SW + (h + 1) * SW // 2],
                    xts_ext[s * TT:(s + 1) * TT, h * SW // 2:(h + 1) * SW // 2],
                )

        # ---- PE warmup: dep-free matmuls so the HAM activity monitor lifts
        # the PE clock to 2.4 GHz before the projection starts ----
        nc.gpsimd.memset(warm_sb[:], 0.0)
        for _ in range(NWARM):
            warm_ps = psum.tile([TT, QS], F32, tag="proj", bufs=1, name="warm_ps")
            nc.tensor.matmul(
                warm_ps[:], warm_sb[:, 0:TT], warm_sb[:], start=True, stop=True,
                skip_group_check=True,
            )

        def proj_ops(ts: int):
            """Emit-closures for projecting super ts (interleave units)."""
            tsl = slice(ts * QS, (ts + 1) * QS)
            ops = []
            pp_box = [None, None]

            def mk_mm(half, c):
                def _f():
                    if c == 0:
                        pp_box[half] = psum.tile([TT, QS], F32, tag="proj",
                                                 bufs=1, name=f"pp{ts}_{half}")
                    nc.tensor.matmul(
                        pp_box[half][:],
                        w_sb[:, c * 2 * TT + half * TT:c * 2 * TT + (half + 1) * TT],
                        xt_sb[:, ts * SW + c * QS:ts * SW + (c + 1) * QS],
                        start=(c == 0),
                        stop=(c == NDT - 1),
                        skip_group_check=True,
                    )
                return _f

            for c in range(NDT):
                ops.append(mk_mm(0, c))

            def qk_evac():
                pp = pp_box[0]
                # Q^T/8 + bias -> rows 0:64, duplicated to rows 64:128
                nc.vector.tensor_scalar_add(
                    qt_sb[0:HS, tsl], pp[0:HS, :], bcol_sb[0:HS, 0:1]
                )
                nc.vector.tensor_copy(qt_sb[HS:2 * HS, tsl], qt_sb[0:HS, tsl])
                # K^T + bias, packed: k-tile 4ts+i -> pair-col u=2ts+i//2,
                # rows 0:64 for even i, 64:128 for odd i
                for i in range(4):
                    u = 2 * ts + i // 2
                    rows = slice(0, HS) if i % 2 == 0 else slice(HS, 2 * HS)
                    nc.vector.tensor_scalar_add(
                        kt_sb[rows, u * TT:(u + 1) * TT],
                        pp[HS:2 * HS, i * TT:(i + 1) * TT],
                        bcol_sb[HS:2 * HS, 0:1],
                    )
            ops.append(qk_evac)

            for c in range(NDT):
                ops.append(mk_mm(1, c))

            vt_box = [None]

            def vt_add():
                vt_box[0] = vt_pool.tile([VP, QS], BF16, tag="vt", name=f"vt{ts}")
                nc.vector.tensor_scalar_add(
                    vt_box[0][0:VP, :], pp_box[1][0:VP, :], bcol_sb[0:VP, 1:2]
                )
            ops.append(vt_add)

            smv_box = [None]

            def mk_vtr(u):
                def _f():
                    if u == 0:
                        # 66-wide slots keep PSUM write offsets 4B-aligned
                        smv_box[0] = psum.tile([TT, 4 * (VP + 1)], BF16,
                                               tag="smv", bufs=1,
                                               name=f"smv{ts}")
                    nc.tensor.transpose(
                        smv_box[0][:, u * (VP + 1):u * (VP + 1) + VP],
                        vt_box[0][:, u * TT:(u + 1) * TT],
                        idb_sb[0:VP, 0:VP],
                    )
                return _f
            for u in range(4):
                ops.append(mk_vtr(u))

            def vp_copy():
                smv3 = smv_box[0][:].rearrange("p (u v) -> p u v", u=4)
                vp3 = vp_sb[:, 4 * ts * VP:(4 * ts + 4) * VP].rearrange(
                    "p (u v) -> p u v", u=4)
                nc.vector.tensor_copy(vp3[:, :, :], smv3[:, :, 0:VP])
            ops.append(vp_copy)
            return ops

        def emit_super(ts: int, filler):
            """Attention for super ts with `filler` ops interleaved; then
            epilogue."""
            tsl = slice(ts * QS, (ts + 1) * QS)
            nkt = 4 * ts + 4
            npair = nkt // 2
            fill_i = [0]

            def emit_fill(frac_done):
                tgt = int(round(frac_done * len(filler)))
                while fill_i[0] < tgt:
                    filler[fill_i[0]]()
                    fill_i[0] += 1

            ot = psum.tile([VP, QS], F32, tag="acc", bufs=1, name=f"ot{ts}")

            def s_pair(p):
                sp = psum.tile([TT, 2 * QS], F32, tag="sbig", bufs=2,
                               name=f"sp{ts}_{p}")
                for h in range(2):
                    jj = 2 * p + h
                    c0 = TT * (jj - 4 * ts) if jj >= 4 * ts else 0
                    rows = slice(0, HS) if h == 0 else slice(HS, 2 * HS)
                    nc.tensor.matmul(
                        sp[:, h * QS + c0:(h + 1) * QS],
                        kt_sb[rows, p * TT:(p + 1) * TT],
                        qt_sb[rows, ts * QS + c0:(ts + 1) * QS],
                        start=True,
                        stop=True,
                        skip_group_check=True,
                    )
                return sp

            def do_exp(p, sp):
                ptile = pt_pool.tile([TT, 2 * QS], BF16, tag="pt",
                                     name=f"pt{ts}_{p}")
                if 2 * p + 1 < 4 * ts:
                    # off-diagonal pair: one activation over both tiles
                    nc.scalar.activation(
                        ptile[:], sp[:], mybir.ActivationFunctionType.Exp
                    )
                else:
                    # diagonal pair: exact written ranges, one per tile
                    for h in range(2):
                        jj = 2 * p + h
                        c0 = TT * (jj - 4 * ts) if jj >= 4 * ts else 0
                        nc.scalar.activation(
                            ptile[:, h * QS + c0:(h + 1) * QS],
                            sp[:, h * QS + c0:(h + 1) * QS],
                            mybir.ActivationFunctionType.Exp,
                        )
                for h in range(2):
                    jj = 2 * p + h
                    if jj >= 4 * ts:
                        b0 = h * QS + TT * (jj - 4 * ts)
                        nc.vector.tensor_mul(
                            ptile[:, b0:b0 + TT], ptile[:, b0:b0 + TT],
                            mask_sb[:],
                        )
                return ptile

            def pv(p, ptile):
                for h in range(2):
                    jj = 2 * p + h
                    c0 = TT * (jj - 4 * ts) if jj >= 4 * ts else 0
                    nc.tensor.matmul(
                        ot[:, c0:QS],
                        vp_sb[:, jj * VP:(jj + 1) * VP],
                        ptile[:, h * QS + c0:(h + 1) * QS],
                        start=(jj == 0),
                        stop=(jj == nkt - 1),
                        skip_group_check=True,
                    )

            sps = {0: s_pair(0)}
            for p in range(npair):
                if p + 1 < npair:
                    sps[p + 1] = s_pair(p + 1)
                ptile = do_exp(p, sps.pop(p))
                pv(p, ptile)
                emit_fill((p + 1) / npair)

            # -- epilogue (fp32): normalize + transpose back + store --
            ot_sb = otsb_pool.tile([VP, QS], F32, tag="otsb")
            nc.vector.tensor_copy(ot_sb[:], ot[:])
            smo = psum.tile([TT, 4 * VP], F32, tag="smo", bufs=1)
            for u in range(4):
                nc.tensor.transpose(
                    smo[:, u * VP:(u + 1) * VP],
                    ot_sb[:, u * TT:(u + 1) * TT],
                    id_sb[0:VP, 0:VP],
                )
            o_sb = osb_pool.tile([TT, 4 * HS], F32, tag="osb")
            rc = rc_pool.tile([TT, 4], F32, tag="rc")
            smo3 = smo[:].rearrange("p (u v) -> p u v", u=4)
            rc3 = rc[:].rearrange("p (u v) -> p u v", v=1)
            nc.vector.reciprocal(rc3[:, :, :], smo3[:, :, HS:HS + 1])
            for u in range(4):
                nc.vector.tensor_scalar_mul(
                    o_sb[:, u * HS:(u + 1) * HS],
                    smo[:, u * VP:u * VP + HS],
                    rc[:, u:u + 1],
                )
            nc.sync.dma_start(
                out_ext[tsl, :].rearrange("(u p) h -> p u h", p=TT),
                o_sb[:].rearrange("p (u h) -> p u h", u=4),
            )

        # super 0 projection up front, then attention(ts) with proj(ts+1)
        # interleaved
        for op in proj_ops(0):
            op()
        for ts in range(NQS):
            filler = proj_ops(ts + 1) if ts + 1 < NQS else []
            emit_super(ts, filler)

    nc.compile()
    return nc


def make_inputs(x_b, Wq, bq, Wk, bk, Wv, bv):
    """Host-side prep for one core's in_map (x_b: [T, D] fp32)."""
    import ml_dtypes

    bf = ml_dtypes.bfloat16
    scale = 1.0 / np.sqrt(np.float32(HS))
    w = np.zeros((D, 2 * TT), dtype=np.float32)
    w[:, 0:HS] = Wq * scale
    w[:, HS:2 * HS] = Wk
    w[:, 2 * HS:3 * HS] = Wv
    # swizzle: w_sb[p, c*256+j] = w[c*128+p, j]
    wsw = np.ascontiguousarray(
        w.reshape(NDT, TT, 2 * TT).transpose(1, 0, 2).reshape(TT, NDT * 2 * TT)
    )
    bcol = np.zeros((TT, 2), dtype=np.float32)
    bcol[0:HS, 0] = bq * scale
    bcol[HS:2 * HS, 0] = bk
    bcol[0:HS, 1] = bv
    bcol[HS, 1] = 1.0
    # xts[s*128+p, c*512+q] = x_b[s*512+q, c*128+p]
    xts = np.ascontiguousarray(
        x_b.reshape(NQS, QS, NDT, TT).transpose(0, 3, 2, 1)
        .reshape(NQS * TT, NDT * QS)
    ).astype(bf)
    mask = np.triu(np.ones((TT, TT), dtype=bf))
    return {
        "xts": xts,
        "wqkv": wsw.astype(bf),
        "bcol": bcol,
        "ident": np.eye(TT, dtype=np.float32),
        "identb": np.eye(TT, dtype=bf),
        "mask": mask,
    }


_NC_CACHE = None


def _get_nc():
    global _NC_CACHE
    if _NC_CACHE is None:
        _NC_CACHE = build_graph()
    return _NC_CACHE


def kernel(x, Wq, bq, Wk, bk, Wv, bv):
    x = np.asarray(x, dtype=np.float32)
    args = [np.asarray(a, dtype=np.float32) for a in (Wq, bq, Wk, bk, Wv, bv)]
    nc = _get_nc()
    in_maps = [make_inputs(x[b], *args) for b in range(N_CORES)]
    trace = os.environ.get("BASS_ATTN_TRACE", "0") == "1"
    res = run_bass_kernel_spmd(
        nc, in_maps, core_ids=list(range(N_CORES)), trace=trace
    )
    if trace:
        print(
            f"HW exec time: {res.exec_time_ns} ns "
            f"(mean {res.mean_exec_time_ns}, max core {res.max_exec_time_core_id})"
        )
    out = np.stack([res.results[b]["out"] for b in range(N_CORES)], axis=0)
    return out
